# revision 14
# baseline (speedup 1.0000x reference)
"""Trainium2 Bass kernel for ApertureChamberSSM (v11, fp8 DoubleRow scan).

Computation (reference):
    iv, ov, beta_s, alpha, mg = sigmoid(scalars); decay = exp(-alpha)
    x_in  = iv * x ; drive = tanh(x_in)
    psi_s = decay * psi_{s-1} + (1-decay) * drive_s          (scan over S)
    x_mem = mg * psi + (1-mg) * x_in
    rotate channel pairs (j, j+512) by pi*sigmoid(beta), scale by ov

Algebra: psi = (1-decay)*psi' with psi'_s = decay*psi'_{s-1} + drive_s
    out = a_*R@psi' + c*R@x,  a_ = mg*(1-decay), c = (1-mg)*iv,
    R = ov*[[cos,-sin],[sin,cos]].

Matmul-scan: decay^64 ~ 3e-9, so the scan history is < 64 steps.  Sequence
positions go on the partition axis in 64-position blocks (partition =
2*t + {re,im}); the scan is a dense matmul with a lower-triangular
decay-Toeplitz matrix T, the cross-block carry is a second Toeplitz Tc
reading the previous block's drive, the pair rotation folds into the
weights as a Kronecker factor, and the c*x passthrough is a block-diag
matmul.  PSUM accumulates the finished output; no serial scan anywhere:

    out_blk = [a_*(T (x) R)] @ drive_blk          T[p,t]  = decay^(p-t), p>=t
            + [a_*(Tc (x) R)] @ drive_{blk-1}     Tc[p,t] = decay^(p+64-t)
            + [c*(I (x) R)] @ x_blk

fp8 DoubleRow (v11): drive is stored fp8-e4m3 and the T and Tc matmuls
fuse into ONE DoubleRow fp8 matmul — the PE consumes two k-rows per
cycle: out[m,n] = sum_p sum_i w[p,i,m] x[p,i,n], with k-tile i=0 the
carry columns and i=1 the current columns.  The drive tile's 64-column
zero/copy prefix makes both k-tiles strided views of one tile (k-tile
stride = 64 columns).  fp8 weights are pre-scaled by S8 ~ 232/max|W| to
use the full e4m3 range (keeps all 64 decay taps normal); the 1/S8
rescale rides the PSUM eviction for free (tensor_scalar costs the same
as tensor_copy).  Measured end-to-end rel err ~7.5e-3 (gate 2e-2).
PE per 512 output cols: 2 matmuls (was 3).

Engines: ACT tanh -> fp8, DVE PSUM evictions (x 1/S8), PE warmup + 2
matmuls per 512 cols, sync HWDGE input ring, stores alternate scalar
HWDGE / gpsimd SWDGE rings (one ring tops out ~200 GB/s; two track the
~400 GB/s core cap), gpsimd prefix copies.  First-chunk DMAs taper; the
final chunk drains in 512-col pieces alternating ACT/DVE + both store
rings.

Layout: per core 64 channel pairs (j, j+512), j in [64c, 64c+64).  DRAM
x/out are [128, 32768]: partition = 2*(s % 64) + {0:re,1:im}, column =
batch*8192 + (s//64)*64 + pair.  8 cores, zero comms.
"""

import math

import numpy as np

B, S, D = 4, 8192, 1024
HALF = D // 2           # 512
NCORES = 8
JPC = HALF // NCORES    # 64 channel pairs per core
P = 128                 # partitions
TB = P // 2             # 64 sequence positions per block
NB = S // TB            # 128 blocks per batch
CB = NB * JPC           # 8192 columns per batch
F = B * CB              # 32768 columns per core
C = 4096                # columns per chunk
NCHUNK = F // C         # 8
CPB = CB // C           # 2 chunks per batch
MMF = 512               # matmul moving free dim (one PSUM bank)
EG = 2048               # eviction / psum-tile / store granularity
NWARM = 32              # dummy matmuls to warm the PE HAM clock gate

_cache = {}


def _sig(v):
    return 1.0 / (1.0 + math.exp(-float(v)))


def _build(tanh_scale, s8):
    import concourse.bass as bass
    import concourse.tile as tile
    from concourse import bacc, mybir
    from concourse.ap import AP

    f32 = mybir.dt.float32
    bf16 = mybir.dt.bfloat16
    f8 = mybir.dt.float8e4
    AF = mybir.ActivationFunctionType
    DR = mybir.MatmulPerfMode.DoubleRow
    inv_s8 = 1.0 / s8

    nc = bacc.Bacc("TRN2", target_bir_lowering=False, debug=False,
                   num_devices=NCORES)
    x_ap = nc.dram_tensor("x", [P, F], bf16, kind="ExternalInput").ap()
    c8_ap = nc.dram_tensor("consts8", [P, 2, P], f8,
                           kind="ExternalInput").ap()
    c16_ap = nc.dram_tensor("consts", [P, P], bf16,
                            kind="ExternalInput").ap()
    out_ap = nc.dram_tensor("out", [P, F], bf16, kind="ExternalOutput").ap()

    with tile.TileContext(nc) as tc:
        with (
            tc.tile_pool(name="const", bufs=1) as cpool,
            tc.tile_pool(name="xin", bufs=5) as xpool,
            tc.tile_pool(name="drv", bufs=5) as dpool,
            tc.tile_pool(name="outs", bufs=5) as opool,
            tc.tile_pool(name="ps", bufs=1, space=bass.MemorySpace.PSUM) as pspool,
        ):
            # consts ride the scalar HWDGE ring: keeps the sync ring free
            # so the first x DMA issues earlier
            w8 = cpool.tile([P, 2, P], f8, tag="w8")
            nc.scalar.dma_start(w8[:, :, :], c8_ap[:, :, :])
            w3 = cpool.tile([P, P], bf16, tag="w3")
            nc.scalar.dma_start(w3[:], c16_ap[:])
            W12 = w8[:, 0:2, :]       # [(a_*(Tc (x) R)).T ; (a_*(T (x) R)).T]
            W3 = w3[:, 0:P]           # (c *(I  (x) R)).T

            # warmup, off the critical path: ~3.4us of PE activity flips the
            # HAM clock gate to 2.4 GHz and a dummy activation pulls the ACT
            # table load forward; both depend only on one memset.
            dum = cpool.tile([P, 2 * TB], bf16, tag="dum")
            nc.vector.memset(dum[:], 0.0078125)
            ps_w = pspool.tile([P, EG], f32, tag="ps0")
            for _ in range(NWARM):
                nc.tensor.matmul(ps_w[0:TB, 0:TB], dum[:, 0:TB],
                                 dum[:, 0:TB], start=True, stop=True)
            nc.scalar.activation(dum[:, TB:2 * TB], dum[:, 0:TB],
                                 AF.Tanh, bias=0.0, scale=1.0)

            prev_d = [None]
            nevict = [0]

            def drv_ap(d_t, c0):
                """[128, 2, MMF] view of d_t: ktile 0 = carry cols (c0..),
                ktile 1 = current cols (c0+TB..)."""
                base = d_t[:, 0:1]
                return AP(base.tensor, base.offset + c0,
                          [list(base.ap[0]), [TB, 2], [1, MMF]])

            def front(k):
                x_t = xpool.tile([P, C], bf16, tag="x")
                d_t = dpool.tile([P, TB + C], f8, tag="d")
                if k == 0:      # taper so tanh/matmul start early
                    pieces = [512] * 4 + [1024] * ((C - 2048) // 1024)
                elif k == 1:
                    pieces = [1024] * (C // 1024)
                else:
                    pieces = [2048] * (C // 2048)
                off = 0
                for pi, w in enumerate(pieces):
                    sl = slice(off, off + w)
                    # ramp phase (before stores start): odd pieces ride the
                    # gpsimd SWDGE ring — the sync ring alone tops out
                    # ~300 GB/s input-only
                    ieng = nc.gpsimd if (k < 3 and pi % 2 == 1) else nc.sync
                    ieng.dma_start(x_t[:, sl],
                                   x_ap[:, k * C + off:k * C + off + w])
                    nc.scalar.activation(d_t[:, TB + off:TB + off + w],
                                         x_t[:, sl], AF.Tanh,
                                         bias=0.0, scale=tanh_scale)
                    off += w
                # prefix ops go on the otherwise-idle GpSimd engine: on the
                # strict-FIFO DVE queue they would head-of-line block the
                # PSUM evictions behind them
                if k % CPB == 0:
                    nc.gpsimd.memset(d_t[:, 0:TB], 0.0)  # batch start
                else:
                    nc.gpsimd.tensor_copy(d_t[:, 0:TB],
                                          prev_d[0][:, C:TB + C])
                return x_t, d_t

            def back(k, x_t, d_t):
                o_t = opool.tile([P, C], bf16, tag="o")
                last = k >= NCHUNK - 2
                for t in range(C // EG):
                    ps = pspool.tile([P, EG], f32,
                                     tag=f"ps{(k * (C // EG) + t) % 2}")
                    for g in range(EG // MMF):
                        c0 = t * EG + g * MMF
                        fo = slice(g * MMF, (g + 1) * MMF)
                        nc.tensor.matmul(ps[:, fo], W12, drv_ap(d_t, c0),
                                         start=True, stop=False,
                                         perf_mode=DR)
                        nc.tensor.matmul(ps[:, fo], W3, x_t[:, c0:c0 + MMF],
                                         start=False, stop=True,
                                         skip_group_check=True)
                    if last:
                        # fine-grained drain of the whole final chunk,
                        # alternating ACT/DVE evictions and scalar/gpsimd
                        # store rings so the tail parallelizes
                        for g in range(EG // MMF):
                            osl = slice(t * EG + g * MMF,
                                        t * EG + (g + 1) * MMF)
                            fo = slice(g * MMF, (g + 1) * MMF)
                            if g % 2 == 0:
                                nc.scalar.activation(o_t[:, osl], ps[:, fo],
                                                     AF.Copy, bias=0.0,
                                                     scale=inv_s8)
                            else:
                                nc.vector.tensor_scalar_mul(
                                    o_t[:, osl], ps[:, fo], inv_s8)
                            seng = nc.scalar if g % 2 == 0 else nc.gpsimd
                            seng.dma_start(
                                out_ap[:, k * C + t * EG + g * MMF:
                                       k * C + t * EG + (g + 1) * MMF],
                                o_t[:, osl])
                        continue
                    # all steady-state evictions on DVE: an eviction in the
                    # strict-FIFO ACT queue blocks the next chunk's tanh
                    # behind it, starving the PE two chunks later.  The
                    # 1/S8 rescale rides the eviction for free.  Stores
                    # alternate between the scalar HWDGE ring and the
                    # gpsimd SWDGE ring: one ring tops out ~200 GB/s, two
                    # track the input rate.
                    nevict[0] += 1
                    osl = slice(t * EG, (t + 1) * EG)
                    nc.vector.tensor_scalar_mul(o_t[:, osl], ps[:], inv_s8)
                    seng = nc.scalar if nevict[0] % 2 == 0 else nc.gpsimd
                    seng.dma_start(
                        out_ap[:, k * C + t * EG:k * C + (t + 1) * EG],
                        o_t[:, osl])

            pend = None
            for k in range(NCHUNK):
                cur = front(k)
                prev_d[0] = cur[1]
                if pend is not None:
                    back(*pend)
                pend = (k, *cur)
            back(*pend)

    nc.compile()
    return nc


def _weights(iv, ov, decay, a_, c, angle):
    """(M1, M2, M3) in float64: out_blk = M1@drive + M2@drive_prev + M3@x."""
    t = np.arange(TB)
    diff = t[:, None] - t[None, :]                  # p - t
    T = np.where(diff >= 0, decay ** np.maximum(diff, 0), 0.0)
    Tc = decay ** (diff + TB)
    R = ov * np.array([[math.cos(angle), -math.sin(angle)],
                       [math.sin(angle), math.cos(angle)]])
    M1 = a_ * np.kron(T, R)
    M2 = a_ * np.kron(Tc, R)
    M3 = c * np.kron(np.eye(TB), R)
    return M1, M2, M3


def kernel(x, beta, input_valve, output_valve, alpha_raw, memory_gate):
    x = np.asarray(x, dtype=np.float32)
    assert x.shape == (B, S, D), x.shape

    beta_s = _sig(beta)
    iv = _sig(input_valve)
    ov = _sig(output_valve)
    alpha = _sig(alpha_raw)
    mg = _sig(memory_gate)
    decay = math.exp(-alpha)
    c = (1.0 - mg) * iv
    a_ = mg * (1.0 - decay)
    angle = math.pi * beta_s

    M1, M2, M3 = _weights(iv, ov, decay, a_, c, angle)
    s8 = 232.0 / max(np.abs(M1).max(), np.abs(M2).max(), 1e-30)

    key = (round(iv, 12), round(s8, 6))
    if key not in _cache:
        _cache[key] = _build(iv, s8)
    nc = _cache[key]

    import ml_dtypes
    from concourse.bass_utils import run_bass_kernel_spmd

    bf = ml_dtypes.bfloat16
    f8 = ml_dtypes.float8_e4m3
    consts8 = np.stack([np.clip((M2 * s8).T, -240, 240),
                        np.clip((M1 * s8).T, -240, 240)],
                       axis=1).astype(f8)          # (P, 2, P)
    consts16 = ((M3 * s8).T).astype(bf)            # (P, P)

    # pack: partition = 2*(s%64) + {0:re,1:im}; col = b*8192 + (s//64)*64 + jp
    in_maps = []
    for cix in range(NCORES):
        shard = np.empty((B, P, CB), dtype=bf)
        for b in range(B):
            vr = x[b][:, 64 * cix:64 * cix + JPC].reshape(NB, TB, JPC)
            vi = x[b][:, HALF + 64 * cix:HALF + 64 * cix + JPC].reshape(
                NB, TB, JPC)
            st = np.stack([vr, vi], axis=2)          # (NB, TB, 2, JPC)
            shard[b] = st.transpose(1, 2, 0, 3).reshape(P, CB).astype(bf)
        in_maps.append({"x": shard.transpose(1, 0, 2).reshape(P, F),
                        "consts8": consts8, "consts": consts16})

    res = run_bass_kernel_spmd(nc, in_maps, core_ids=list(range(NCORES)))
    global last_result
    last_result = res

    out = np.empty((B, S, D), dtype=np.float32)
    for cix in range(NCORES):
        oc = np.asarray(res.results[cix]["out"]).reshape(P, B, CB)
        for b in range(B):
            st = oc[:, b, :].reshape(TB, 2, NB, JPC).transpose(2, 0, 1, 3)
            out[b, :, 64 * cix:64 * cix + JPC] = \
                st[:, :, 0, :].reshape(S, JPC).astype(np.float32)
            out[b, :, HALF + 64 * cix:HALF + 64 * cix + JPC] = \
                st[:, :, 1, :].reshape(S, JPC).astype(np.float32)
    return out


# revision 17
# speedup vs baseline: 1.1763x; 1.1763x over previous
"""Trainium2 Bass kernel for ApertureChamberSSM (v11, fp8 DoubleRow scan).

Computation (reference):
    iv, ov, beta_s, alpha, mg = sigmoid(scalars); decay = exp(-alpha)
    x_in  = iv * x ; drive = tanh(x_in)
    psi_s = decay * psi_{s-1} + (1-decay) * drive_s          (scan over S)
    x_mem = mg * psi + (1-mg) * x_in
    rotate channel pairs (j, j+512) by pi*sigmoid(beta), scale by ov

Algebra: psi = (1-decay)*psi' with psi'_s = decay*psi'_{s-1} + drive_s
    out = a_*R@psi' + c*R@x,  a_ = mg*(1-decay), c = (1-mg)*iv,
    R = ov*[[cos,-sin],[sin,cos]].

Matmul-scan: decay^64 ~ 3e-9, so the scan history is < 64 steps.  Sequence
positions go on the partition axis in 64-position blocks (partition =
2*t + {re,im}); the scan is a dense matmul with a lower-triangular
decay-Toeplitz matrix T, the cross-block carry is a second Toeplitz Tc
reading the previous block's drive, the pair rotation folds into the
weights as a Kronecker factor, and the c*x passthrough is a block-diag
matmul.  PSUM accumulates the finished output; no serial scan anywhere:

    out_blk = [a_*(T (x) R)] @ drive_blk          T[p,t]  = decay^(p-t), p>=t
            + [a_*(Tc (x) R)] @ drive_{blk-1}     Tc[p,t] = decay^(p+64-t)
            + [c*(I (x) R)] @ x_blk

fp8 DoubleRow (v11): drive is stored fp8-e4m3 and the T and Tc matmuls
fuse into ONE DoubleRow fp8 matmul — the PE consumes two k-rows per
cycle: out[m,n] = sum_p sum_i w[p,i,m] x[p,i,n], with k-tile i=0 the
carry columns and i=1 the current columns.  The drive tile's 64-column
zero/copy prefix makes both k-tiles strided views of one tile (k-tile
stride = 64 columns).  fp8 weights are pre-scaled by S8 ~ 232/max|W| to
use the full e4m3 range (keeps all 64 decay taps normal); the 1/S8
rescale rides the PSUM eviction for free (tensor_scalar costs the same
as tensor_copy).  Measured end-to-end rel err ~7.5e-3 (gate 2e-2).
PE per 512 output cols: 2 matmuls (was 3).

Engines: ACT tanh -> fp8, DVE PSUM evictions (x 1/S8), PE warmup + 2
matmuls per 512 cols, sync HWDGE input ring, stores alternate scalar
HWDGE / gpsimd SWDGE rings (one ring tops out ~200 GB/s; two track the
~400 GB/s core cap), gpsimd prefix copies.  First-chunk DMAs taper; the
final chunk drains in 512-col pieces alternating ACT/DVE + both store
rings.

Layout: per core 64 channel pairs (j, j+512), j in [64c, 64c+64).  DRAM
x/out are [128, 32768]: partition = 2*(s % 64) + {0:re,1:im}, column =
batch*8192 + (s//64)*64 + pair.  8 cores, zero comms.
"""

import math

import numpy as np

B, S, D = 4, 8192, 1024
HALF = D // 2           # 512
NCORES = 8
JPC = HALF // NCORES    # 64 channel pairs per core
P = 128                 # partitions
TB = P // 2             # 64 sequence positions per block
NB = S // TB            # 128 blocks per batch
CB = NB * JPC           # 8192 columns per batch
F = B * CB              # 32768 columns per core
C = 4096                # columns per chunk
NCHUNK = F // C         # 8
CPB = CB // C           # 2 chunks per batch
MMF = 512               # matmul moving free dim (one PSUM bank)
EG = 2048               # eviction / psum-tile / store granularity
NWARM = 32              # dummy matmuls to warm the PE HAM clock gate

_cache = {}


def _sig(v):
    return 1.0 / (1.0 + math.exp(-float(v)))


def _build(tanh_scale, s8):
    import concourse.bass as bass
    import concourse.tile as tile
    from concourse import bacc, mybir
    from concourse.ap import AP

    f32 = mybir.dt.float32
    bf16 = mybir.dt.bfloat16
    f8 = mybir.dt.float8e4
    AF = mybir.ActivationFunctionType
    DR = mybir.MatmulPerfMode.DoubleRow
    inv_s8 = 1.0 / s8

    nc = bacc.Bacc("TRN2", target_bir_lowering=False, debug=False,
                   num_devices=NCORES)
    x_ap = nc.dram_tensor("x", [P, F], bf16, kind="ExternalInput").ap()
    c8_ap = nc.dram_tensor("consts8", [P, 2, P], f8,
                           kind="ExternalInput").ap()
    c16_ap = nc.dram_tensor("consts", [P, P], bf16,
                            kind="ExternalInput").ap()
    out_ap = nc.dram_tensor("out", [P, F], bf16, kind="ExternalOutput").ap()

    with tile.TileContext(nc) as tc:
        with (
            tc.tile_pool(name="const", bufs=1) as cpool,
            tc.tile_pool(name="xin", bufs=5) as xpool,
            tc.tile_pool(name="drv", bufs=5) as dpool,
            tc.tile_pool(name="outs", bufs=4) as opool,
            tc.tile_pool(name="ps", bufs=1, space=bass.MemorySpace.PSUM) as pspool,
        ):
            # consts ride the scalar HWDGE ring: keeps the sync ring free
            # so the first x DMA issues earlier
            w8 = cpool.tile([P, 2, P], f8, tag="w8")
            nc.scalar.dma_start(w8[:, :, :], c8_ap[:, :, :])
            w3 = cpool.tile([P, P], bf16, tag="w3")
            nc.scalar.dma_start(w3[:], c16_ap[:])
            W12 = w8[:, 0:2, :]       # [(a_*(Tc (x) R)).T ; (a_*(T (x) R)).T]
            W3 = w3[:, 0:P]           # (c *(I  (x) R)).T

            # warmup, off the critical path: ~3.4us of PE activity flips the
            # HAM clock gate to 2.4 GHz and a dummy activation pulls the ACT
            # table load forward; both depend only on one memset.
            dum = cpool.tile([P, 2 * TB], bf16, tag="dum")
            nc.vector.memset(dum[:], 0.0078125)
            ps_w = pspool.tile([P, EG], f32, tag="ps0")
            for _ in range(NWARM):
                nc.tensor.matmul(ps_w[0:TB, 0:TB], dum[:, 0:TB],
                                 dum[:, 0:TB], start=True, stop=True)
            nc.scalar.activation(dum[:, TB:2 * TB], dum[:, 0:TB],
                                 AF.Tanh, bias=0.0, scale=1.0)

            prev_d = [None]
            nevict = [0]

            def drv_ap(d_t, c0):
                """[128, 2, MMF] view of d_t: ktile 0 = carry cols (c0..),
                ktile 1 = current cols (c0+TB..)."""
                base = d_t[:, 0:1]
                return AP(base.tensor, base.offset + c0,
                          [list(base.ap[0]), [TB, 2], [1, MMF]])

            def front(k):
                x_t = xpool.tile([P, C], bf16, tag="x")
                d_t = dpool.tile([P, TB + C], f8, tag="d")
                if k == 0:      # taper so tanh/matmul start early
                    pieces = [512] * 4 + [1024] * ((C - 2048) // 1024)
                elif k == 1:
                    pieces = [1024] * (C // 1024)
                else:
                    pieces = [2048] * (C // 2048)
                off = 0
                for pi, w in enumerate(pieces):
                    sl = slice(off, off + w)
                    # ramp phase (before stores start): odd pieces ride the
                    # gpsimd SWDGE ring — the sync ring alone tops out
                    # ~300 GB/s input-only
                    ieng = nc.gpsimd if (k < 3 and pi % 2 == 1) else nc.sync
                    ieng.dma_start(x_t[:, sl],
                                   x_ap[:, k * C + off:k * C + off + w])
                    nc.scalar.activation(d_t[:, TB + off:TB + off + w],
                                         x_t[:, sl], AF.Tanh,
                                         bias=0.0, scale=tanh_scale)
                    off += w
                # prefix ops go on the otherwise-idle GpSimd engine: on the
                # strict-FIFO DVE queue they would head-of-line block the
                # PSUM evictions behind them
                if k % CPB == 0:
                    nc.gpsimd.memset(d_t[:, 0:TB], 0.0)  # batch start
                else:
                    nc.gpsimd.tensor_copy(d_t[:, 0:TB],
                                          prev_d[0][:, C:TB + C])
                return x_t, d_t

            def back(k, x_t, d_t):
                o_t = opool.tile([P, C], bf16, tag="o")
                last = k == NCHUNK - 1
                for t in range(C // EG):
                    ps = pspool.tile([P, EG], f32,
                                     tag=f"ps{(k * (C // EG) + t) % 2}")
                    for g in range(EG // MMF):
                        c0 = t * EG + g * MMF
                        fo = slice(g * MMF, (g + 1) * MMF)
                        nc.tensor.matmul(ps[:, fo], W12, drv_ap(d_t, c0),
                                         start=True, stop=False,
                                         perf_mode=DR)
                        nc.tensor.matmul(ps[:, fo], W3, x_t[:, c0:c0 + MMF],
                                         start=False, stop=True,
                                         skip_group_check=True)
                    if last:
                        # fine-grained drain of the whole final chunk,
                        # alternating ACT/DVE evictions and scalar/gpsimd
                        # store rings so the tail parallelizes
                        for g in range(EG // MMF):
                            osl = slice(t * EG + g * MMF,
                                        t * EG + (g + 1) * MMF)
                            fo = slice(g * MMF, (g + 1) * MMF)
                            if g % 2 == 0:
                                nc.scalar.activation(o_t[:, osl], ps[:, fo],
                                                     AF.Copy, bias=0.0,
                                                     scale=inv_s8)
                            else:
                                nc.vector.tensor_scalar_mul(
                                    o_t[:, osl], ps[:, fo], inv_s8)
                            seng = nc.scalar if g % 2 == 0 else nc.gpsimd
                            seng.dma_start(
                                out_ap[:, k * C + t * EG + g * MMF:
                                       k * C + t * EG + (g + 1) * MMF],
                                o_t[:, osl])
                        continue
                    # all steady-state evictions on DVE: an eviction in the
                    # strict-FIFO ACT queue blocks the next chunk's tanh
                    # behind it, starving the PE two chunks later.  The
                    # 1/S8 rescale rides the eviction for free.  Stores
                    # alternate between the scalar HWDGE ring and the
                    # gpsimd SWDGE ring: one ring tops out ~200 GB/s, two
                    # track the input rate.
                    nevict[0] += 1
                    osl = slice(t * EG, (t + 1) * EG)
                    nc.vector.tensor_scalar_mul(o_t[:, osl], ps[:], inv_s8)
                    if t == C // EG - 1:
                        # one whole-chunk store after the last eviction:
                        # halves the issue count, 8KB rows
                        seng = nc.scalar if k % 2 == 0 else nc.gpsimd
                        seng.dma_start(out_ap[:, k * C:(k + 1) * C], o_t[:])

            pend = None
            for k in range(NCHUNK):
                cur = front(k)
                prev_d[0] = cur[1]
                if pend is not None:
                    back(*pend)
                pend = (k, *cur)
            back(*pend)

    nc.compile()
    return nc


def _weights(iv, ov, decay, a_, c, angle):
    """(M1, M2, M3) in float64: out_blk = M1@drive + M2@drive_prev + M3@x."""
    t = np.arange(TB)
    diff = t[:, None] - t[None, :]                  # p - t
    T = np.where(diff >= 0, decay ** np.maximum(diff, 0), 0.0)
    Tc = decay ** (diff + TB)
    R = ov * np.array([[math.cos(angle), -math.sin(angle)],
                       [math.sin(angle), math.cos(angle)]])
    M1 = a_ * np.kron(T, R)
    M2 = a_ * np.kron(Tc, R)
    M3 = c * np.kron(np.eye(TB), R)
    return M1, M2, M3


def kernel(x, beta, input_valve, output_valve, alpha_raw, memory_gate):
    x = np.asarray(x, dtype=np.float32)
    assert x.shape == (B, S, D), x.shape

    beta_s = _sig(beta)
    iv = _sig(input_valve)
    ov = _sig(output_valve)
    alpha = _sig(alpha_raw)
    mg = _sig(memory_gate)
    decay = math.exp(-alpha)
    c = (1.0 - mg) * iv
    a_ = mg * (1.0 - decay)
    angle = math.pi * beta_s

    M1, M2, M3 = _weights(iv, ov, decay, a_, c, angle)
    s8 = 232.0 / max(np.abs(M1).max(), np.abs(M2).max(), 1e-30)

    key = (round(iv, 12), round(s8, 6))
    if key not in _cache:
        _cache[key] = _build(iv, s8)
    nc = _cache[key]

    import ml_dtypes
    from concourse.bass_utils import run_bass_kernel_spmd

    bf = ml_dtypes.bfloat16
    f8 = ml_dtypes.float8_e4m3
    consts8 = np.stack([np.clip((M2 * s8).T, -240, 240),
                        np.clip((M1 * s8).T, -240, 240)],
                       axis=1).astype(f8)          # (P, 2, P)
    consts16 = ((M3 * s8).T).astype(bf)            # (P, P)

    # pack: partition = 2*(s%64) + {0:re,1:im}; col = b*8192 + (s//64)*64 + jp
    in_maps = []
    for cix in range(NCORES):
        shard = np.empty((B, P, CB), dtype=bf)
        for b in range(B):
            vr = x[b][:, 64 * cix:64 * cix + JPC].reshape(NB, TB, JPC)
            vi = x[b][:, HALF + 64 * cix:HALF + 64 * cix + JPC].reshape(
                NB, TB, JPC)
            st = np.stack([vr, vi], axis=2)          # (NB, TB, 2, JPC)
            shard[b] = st.transpose(1, 2, 0, 3).reshape(P, CB).astype(bf)
        in_maps.append({"x": shard.transpose(1, 0, 2).reshape(P, F),
                        "consts8": consts8, "consts": consts16})

    res = run_bass_kernel_spmd(nc, in_maps, core_ids=list(range(NCORES)))
    global last_result
    last_result = res

    out = np.empty((B, S, D), dtype=np.float32)
    for cix in range(NCORES):
        oc = np.asarray(res.results[cix]["out"]).reshape(P, B, CB)
        for b in range(B):
            st = oc[:, b, :].reshape(TB, 2, NB, JPC).transpose(2, 0, 1, 3)
            out[b, :, 64 * cix:64 * cix + JPC] = \
                st[:, :, 0, :].reshape(S, JPC).astype(np.float32)
            out[b, :, HALF + 64 * cix:HALF + 64 * cix + JPC] = \
                st[:, :, 1, :].reshape(S, JPC).astype(np.float32)
    return out
